# revision 8
# baseline (speedup 1.0000x reference)
"""Trainium2 Bass kernel for ClaimValidationLoss.

Data-parallel over 8 NeuronCores: each core takes 32 of the 256 batches
(32768 claims, 32MB adjacency shard).

The baseline gathered one probability per claim via GPSIMD indirect DMAs
([128,1] out = 128 descriptors/instruction), paying the ~1us SWDGE fixed
overhead 256 times (~270us serialized on the Pool engine). This kernel
instead uses the SWDGE dma_gather ucode (InstDMAGatherAnt), which packs
up to num_idxs descriptors into ONE Pool instruction (994ns + 0.34ns/desc)
at 256-byte granularity:

  * Per-core flat claim offset o = b_local*2^18 + va*512 + vb. dma_gather
    indices are int16 (<32768) over 64-float blocks, so one gather can
    address 2^21 elements = exactly 8 batches. The window split is STATIC:
    window w = local batches [8w, 8w+8), 8192 claims each.
  * idx = va*8 + (vb2 >> 6) where vb2 = vb + 2^18*(batch-within-window)
    (static offset folded on the host); max = 511*8 + 4095 + 7*4096 = 32767.
  * 4 dma_gather instructions (single_packet=False; the 64KB single-packet
    SDMA limit hangs at >4096 descriptors) -> G_w[128, 64, 64] f32, claim
    slot i at partition i%128, group i//128.
  * The claim's element sits at position c = vb & 63 inside its block.
    Extraction = 6 in-place copy_predicated halving steps on DVE (binary
    select on the bits of c), ~4us/window, leaving the value at [:, :, 0].
  * BCE coefficients (s, w with q = p*w + s covering flip / rt>=4 / padding)
    prep on DVE under the gather shadow, Ln on ACT, fp32 reduces, and the
    host all-reduces the per-partition (sum_log_q, n_valid) pairs.
"""

import numpy as np

import concourse.bass as bass  # noqa: F401  (IndirectOffsetOnAxis unused now)
from concourse import bacc, mybir
from concourse.bass_utils import run_bass_kernel_spmd

# Problem geometry (hardcoded per contest contract).
B, N, M = 256, 512, 1024
NCORES = 8
BL = B // NCORES            # 32 batches per core
P = 128                     # SBUF partitions
TC = BL * M                 # 32768 claims per core
NW = 4                      # gather windows (8 batches each; int16 idx limit)
CW = TC // NW               # 8192 claims per window
GW = CW // P                # 64 claim groups per window
CF = NW * GW                # 256 claim columns in [128, 256] layout
NBLK = 32768                # 64-elem blocks per window
EPS = float(np.float32(1e-7))
ONE_M_EPS = float(np.float32(1.0 - 1e-7))

f32 = mybir.dt.float32
i32 = mybir.dt.int32
i16 = mybir.dt.int16
Alu = mybir.AluOpType
Act = mybir.ActivationFunctionType

_CACHE = {}


def _build_nc():
    nc = bacc.Bacc("TRN2", target_bir_lowering=False, debug=False,
                   dynamic_dma_scratch_size=65536)

    # Per-window adjacency shards: [32768 blocks, 64 floats].
    adjw = [nc.dram_tensor(f"adj{w}", [NBLK, 64], f32, kind="ExternalInput")
            for w in range(NW)]
    # Claim meta in window-slot layout [128, 4*CF]: vb | rt | tt | mk,
    # claim i of window w at (p = i%128, col = GW*w + i//128).
    claims = nc.dram_tensor("claims", [P, 4 * CF], i32, kind="ExternalInput")
    # va and vb2 in the SWDGE wrapped layout, host-replicated to all 8
    # GPSIMD core groups: claim i of window w at (16c + i%16, 128*w + i//16).
    wrap = nc.dram_tensor("wrap", [P, 2 * (TC // 16)], i32,
                          kind="ExternalInput")
    out = nc.dram_tensor("out", [P, 2], f32, kind="ExternalOutput")

    cl = nc.alloc_sbuf_tensor("cl", [P, 4 * CF], i32)
    wr = nc.alloc_sbuf_tensor("wr", [P, 2 * (TC // 16)], i32)
    t1 = nc.alloc_sbuf_tensor("t1", [P, TC // 16], i32)
    idxr = nc.alloc_sbuf_tensor("idxr", [P, TC // 16], i16)
    G = nc.alloc_sbuf_tensor("G", [P, NW * GW * 64], f32)
    praw = nc.alloc_sbuf_tensor("praw", [P, CF], f32)
    mb_t = nc.alloc_sbuf_tensor("mb_t", [P, 6 * CF], i32)
    s_tt = nc.alloc_sbuf_tensor("s_tt", [P, CF], f32)
    w_tt = nc.alloc_sbuf_tensor("w_tt", [P, CF], f32)
    q_t = nc.alloc_sbuf_tensor("q_t", [P, CF], f32)
    lg_t = nc.alloc_sbuf_tensor("lg_t", [P, CF], f32)
    ai_t = nc.alloc_sbuf_tensor("ai_t", [P, CF], i32)
    is4_t = nc.alloc_sbuf_tensor("is4_t", [P, CF], i32)
    vf_t = nc.alloc_sbuf_tensor("vf_t", [P, CF], f32)
    consts = nc.alloc_sbuf_tensor("consts", [P, 5], f32)  # .5, 1, 0, eps, 1-eps
    stats = nc.alloc_sbuf_tensor("stats", [P, 2], f32)    # [sum_log_q, n_valid]
    actwarm = nc.alloc_sbuf_tensor("actwarm", [P, 1], f32)

    s_wr = nc.alloc_semaphore("s_wr")       # wrap w0 slice DMA done
    s_wr2 = nc.alloc_semaphore("s_wr2")     # wrap rest DMA done
    s_cl = nc.alloc_semaphore("s_cl")       # claims DMA done
    s_idx = nc.alloc_semaphore("s_idx")     # idx16 ready (w0, then rest)
    s_g = [nc.alloc_semaphore(f"s_g{h}") for h in range(2 * NW)]
    s_vf = nc.alloc_semaphore("s_vf")       # BCE coeffs + n_valid ready
    s_q = nc.alloc_semaphore("s_q")         # per-window q ready
    s_lg = nc.alloc_semaphore("s_lg")       # per-window ln done
    s_ln = nc.alloc_semaphore("s_ln")       # final log reduce done
    s_out = nc.alloc_semaphore("s_out")     # output DMA done

    vb = cl.ap()[:, 0:CF]
    rt = cl.ap()[:, CF:2 * CF]
    tt = cl.ap()[:, 2 * CF:3 * CF]
    mk = cl.ap()[:, 3 * CF:4 * CF]
    vaw = wr.ap()[:, 0:TC // 16]
    vbw = wr.ap()[:, TC // 16:2 * (TC // 16)]

    # ---- input DMAs. Window 0's va/vb slices land first (sync queue) so
    # the first gather prep can start ~6us in; the rest follows on scalar.
    WC = TC // 16 // NW  # 512 wrapped cols per window
    nc.sync.dma_start(wr.ap()[:, 0:WC], wrap.ap()[:, 0:WC]).then_inc(s_wr, 16)
    nc.sync.dma_start(wr.ap()[:, 4 * WC:5 * WC],
                      wrap.ap()[:, 4 * WC:5 * WC]).then_inc(s_wr, 16)
    nc.scalar.dma_start(wr.ap()[:, WC:4 * WC],
                        wrap.ap()[:, WC:4 * WC]).then_inc(s_wr2, 16)
    nc.scalar.dma_start(wr.ap()[:, 5 * WC:8 * WC],
                        wrap.ap()[:, 5 * WC:8 * WC]).then_inc(s_wr2, 16)
    nc.scalar.dma_start(cl.ap()[:, :], claims.ap()[:, :]).then_inc(s_cl, 16)

    # ---- SCALAR: warm the Ln activation table while DMAs run ----
    nc.scalar.activation(out=actwarm.ap()[:, :], in_=actwarm.ap()[:, :],
                         func=Act.Ln, bias=1.0, scale=0.0)   # ln(0*x+1) = 0

    # ---- VECTOR: constants (no deps) ----
    nc.vector.memset(consts.ap()[:, 0:1], 0.5)
    nc.vector.memset(consts.ap()[:, 1:2], 1.0)
    nc.vector.memset(consts.ap()[:, 2:3], 0.0)
    nc.vector.memset(consts.ap()[:, 3:4], EPS)
    nc.vector.memset(consts.ap()[:, 4:5], ONE_M_EPS)

    # ---- VECTOR: gather indices. idx = va*8 + (vb2 >> 6), already in the
    # wrapped+replicated layout, written as int16. Window 0 first so the
    # gather pipeline starts early.
    for part, (lo, hi) in enumerate([(0, WC), (WC, 4 * WC)]):
        vas = wr.ap()[:, lo:hi]
        vbs = wr.ap()[:, 4 * WC + lo:4 * WC + hi]
        nc.vector.wait_ge(s_wr if part == 0 else s_wr2, 32)
        nc.vector.tensor_scalar(out=t1.ap()[:, lo:hi], in0=vbs,
                                scalar1=6, scalar2=None,
                                op0=Alu.arith_shift_right)
        nc.vector.drain()
        nc.vector.scalar_tensor_tensor(out=t1.ap()[:, lo:hi], in0=vas,
                                       scalar=8, in1=t1.ap()[:, lo:hi],
                                       op0=Alu.mult, op1=Alu.add)
        nc.vector.drain()
        nc.vector.tensor_scalar(out=idxr.ap()[:, lo:hi],
                                in0=t1.ap()[:, lo:hi],
                                scalar1=0, scalar2=None, op0=Alu.add)
        nc.vector.maybe_drain_then_inc((s_idx, 1))

    # ---- GPSIMD: eight half-window gathers (4096 descriptors of 256B
    # each) for pipeline granularity; +2 gated on DMA completion of -2.
    HC = CW // 2              # 4096 claims per half
    HW16 = CW // 32           # 256 wrapped idx cols per half
    for h in range(2 * NW):
        w = h // 2
        nc.gpsimd.wait_ge(s_idx, 1 if w == 0 else 2)
        if h >= 2:
            nc.gpsimd.wait_ge(s_g[h - 2], 16)
        nc.gpsimd.dma_gather(
            out_ap=G.ap()[:, h * (GW // 2) * 64:(h + 1) * (GW // 2) * 64]
                    .rearrange("p (g e) -> p g e", e=64),
            in_ap=adjw[w].ap()[:, :],
            idxs_ap=idxr.ap()[:, h * HW16:(h + 1) * HW16],
            num_idxs=HC,
            num_idxs_reg=HC,
            elem_size=64,
            transpose=False,
            single_packet=False,
        ).then_inc(s_g[h], 16)

    # ---- VECTOR: bit masks of c = vb & 63 (for the extraction selects)
    # and BCE coefficient prep, all under the gather shadow.
    nc.vector.wait_ge(s_cl, 16)
    for b in range(6):
        nc.vector.tensor_scalar(out=mb_t.ap()[:, b * CF:(b + 1) * CF],
                                in0=vb, scalar1=1 << b, scalar2=None,
                                op0=Alu.bitwise_and)
    nc.vector.tensor_scalar(out=ai_t.ap()[:, :], in0=rt, scalar1=1,
                            scalar2=None, op0=Alu.bitwise_and)
    nc.vector.tensor_scalar(out=is4_t.ap()[:, :], in0=rt, scalar1=4,
                            scalar2=None, op0=Alu.is_ge)
    nc.vector.tensor_scalar(out=vf_t.ap()[:, :], in0=mk, scalar1=0,
                            scalar2=None, op0=Alu.is_equal)
    nc.vector.drain()
    nc.vector.tensor_tensor(out=s_tt.ap()[:, :], in0=ai_t.ap()[:, :], in1=tt,
                            op=Alu.is_equal)
    nc.vector.tensor_reduce(out=stats.ap()[:, 1:2], in_=vf_t.ap()[:, :],
                            axis=mybir.AxisListType.X, op=Alu.add)
    nc.vector.drain()
    nc.vector.copy_predicated(out=s_tt.ap()[:, :], mask=is4_t.ap()[:, :],
                              data=consts.ap()[:, 0:1].to_broadcast([P, CF]))
    nc.vector.drain()
    nc.vector.tensor_scalar(out=w_tt.ap()[:, :], in0=s_tt.ap()[:, :],
                            scalar1=-2.0, scalar2=1.0,
                            op0=Alu.mult, op1=Alu.add)
    nc.vector.drain()
    nc.vector.copy_predicated(out=w_tt.ap()[:, :], mask=mk,
                              data=consts.ap()[:, 2:3].to_broadcast([P, CF]))
    nc.vector.copy_predicated(out=s_tt.ap()[:, :], mask=mk,
                              data=consts.ap()[:, 1:2].to_broadcast([P, CF]))
    nc.vector.maybe_drain_then_inc((s_vf, 1))

    # ---- VECTOR/SCALAR per half-window: extract claim element via 6
    # halving selects on the bits of c, then q = p*w + s, clamp (via consts
    # tensors -- dual-immediate tensor_scalar stalls against SWDGE preps);
    # ACT does ln(q).
    GH = GW // 2  # 32 claim groups per half
    for h in range(2 * NW):
        G3 = G.ap()[:, h * GH * 64:(h + 1) * GH * 64] \
              .rearrange("p (g e) -> p g e", e=64)
        cols = slice(h * GH, (h + 1) * GH)
        nc.vector.wait_ge(s_g[h], 16)
        for b in range(5, -1, -1):
            hw = 1 << b
            mask = mb_t.ap()[:, b * CF + h * GH:b * CF + (h + 1) * GH] \
                       .unsqueeze(2).to_broadcast([P, GH, hw])
            nc.vector.copy_predicated(out=G3[:, :, 0:hw], mask=mask,
                                      data=G3[:, :, hw:2 * hw])
            nc.vector.drain()
        g0 = G3[:, :, 0:1].squeeze(2)
        if h == 0:
            nc.vector.wait_ge(s_vf, 1)
        nc.vector.tensor_tensor(out=q_t.ap()[:, cols], in0=g0,
                                in1=w_tt.ap()[:, cols], op=Alu.mult)
        nc.vector.drain()
        nc.vector.tensor_tensor(out=q_t.ap()[:, cols],
                                in0=q_t.ap()[:, cols],
                                in1=s_tt.ap()[:, cols], op=Alu.add)
        nc.vector.drain()
        nc.vector.tensor_tensor(out=q_t.ap()[:, cols],
                                in0=q_t.ap()[:, cols],
                                in1=consts.ap()[:, 3:4].to_broadcast([P, GH]),
                                op=Alu.max)
        nc.vector.drain()
        nc.vector.tensor_tensor(out=q_t.ap()[:, cols],
                                in0=q_t.ap()[:, cols],
                                in1=consts.ap()[:, 4:5].to_broadcast([P, GH]),
                                op=Alu.min)
        nc.vector.maybe_drain_then_inc((s_q, 1))

        nc.scalar.wait_ge(s_q, h + 1)
        nc.scalar.activation(out=lg_t.ap()[:, cols], in_=q_t.ap()[:, cols],
                             func=Act.Ln)
        nc.scalar.maybe_drain_then_inc((s_lg, 1))

    # ---- VECTOR: stats[:,0] = sum of ln(q) (full-fp32 DVE reduce) ----
    nc.vector.wait_ge(s_lg, 2 * NW)
    nc.vector.tensor_reduce(out=stats.ap()[:, 0:1], in_=lg_t.ap()[:, :],
                            axis=mybir.AxisListType.X, op=Alu.add)
    nc.vector.maybe_drain_then_inc((s_ln, 1))

    # ---- SYNC: ship per-partition stats; host does the tiny all-reduce ----
    nc.sync.wait_ge(s_ln, 1)
    nc.sync.wait_ge(s_vf, 1)
    nc.sync.dma_start(out.ap()[:, :], stats.ap()[:, :]).then_inc(s_out, 16)
    nc.sync.wait_ge(s_out, 16)

    nc.compile()
    return nc


def _prep_core_inputs(adj, va, vb, rt, tt, mk, c):
    """Build one core's input map (batches [32c, 32c+32))."""
    sl = slice(c * BL, (c + 1) * BL)
    adj_c = adj[sl].reshape(NW, NBLK, 64)

    def claim_layout(F):
        # claim i of window w -> (p = i%128, col = GW*w + i//128)
        X = F[sl].reshape(NW, GW, P)
        return np.concatenate([X[w].T for w in range(NW)], axis=1)

    def wrap_layout(F):
        # claim i of window w -> (i%16, 128*w + i//16), replicated x8
        X = F.reshape(NW, CW // 16, 16)
        W16 = np.concatenate([X[w].T for w in range(NW)], axis=1)
        return np.tile(W16, (8, 1))

    va_c = va[sl].reshape(TC)
    # vb2 folds the static window-local batch offset: claim i belongs to
    # local batch i//1024, i.e. batch (i//1024)%8 of its window.
    vb2_c = vb[sl].reshape(TC) + (1 << 18) * ((np.arange(TC) // M) % (BL // NW))

    in_map = {f"adj{w}": adj_c[w] for w in range(NW)}
    in_map["claims"] = np.concatenate(
        [claim_layout(vb), claim_layout(rt), claim_layout(tt),
         claim_layout(mk)], axis=1)
    in_map["wrap"] = np.concatenate(
        [wrap_layout(va_c), wrap_layout(vb2_c)], axis=1)
    return in_map


def kernel(posterior_adjacency, var_a, var_b, relation_type, is_true,
           claim_mask):
    adj = np.asarray(posterior_adjacency, dtype=np.float32)
    va = np.asarray(var_a, dtype=np.int32)
    vb = np.asarray(var_b, dtype=np.int32)
    rt = np.asarray(relation_type, dtype=np.int32)
    tt = np.asarray(is_true, dtype=np.int32)
    mk = np.asarray(claim_mask).astype(np.int32)

    if "nc" not in _CACHE:
        _CACHE["nc"] = _build_nc()
    nc = _CACHE["nc"]

    in_maps = [_prep_core_inputs(adj, va, vb, rt, tt, mk, c)
               for c in range(NCORES)]

    res = run_bass_kernel_spmd(nc, in_maps, core_ids=list(range(NCORES)))
    pairs = np.stack([r["out"] for r in res.results]).astype(np.float64)
    sum_log_q = pairs[:, :, 0].sum()
    n_valid = pairs[:, :, 1].sum()
    if n_valid > 0:
        loss = -sum_log_q / max(n_valid, 1.0)
    else:
        loss = 0.0
    return np.float32(loss)


# revision 10
# speedup vs baseline: 1.1694x; 1.1694x over previous
"""Trainium2 Bass kernel for ClaimValidationLoss.

Data-parallel over 8 NeuronCores: each core takes 32 of the 256 batches
(32768 claims, 32MB adjacency shard).

The baseline gathered one probability per claim via GPSIMD indirect DMAs
([128,1] out = 128 descriptors/instruction), paying the ~1us SWDGE fixed
overhead 256 times (~270us serialized on the Pool engine). This kernel
instead uses the SWDGE dma_gather ucode (InstDMAGatherAnt), which packs
up to num_idxs descriptors into ONE Pool instruction (994ns + 0.34ns/desc)
at 256-byte granularity:

  * Per-core flat claim offset o = b_local*2^18 + va*512 + vb. dma_gather
    indices are int16 (<32768) over 64-float blocks, so one gather can
    address 2^21 elements = exactly 8 batches. The window split is STATIC:
    window w = local batches [8w, 8w+8), 8192 claims each.
  * idx = va*8 + (vb2 >> 6) where vb2 = vb + 2^18*(batch-within-window)
    (static offset folded on the host); max = 511*8 + 4095 + 7*4096 = 32767.
  * 4 dma_gather instructions (single_packet=False; the 64KB single-packet
    SDMA limit hangs at >4096 descriptors) -> G_w[128, 64, 64] f32, claim
    slot i at partition i%128, group i//128.
  * The claim's element sits at position c = vb & 63 inside its block.
    Extraction = 6 in-place copy_predicated halving steps on DVE (binary
    select on the bits of c), ~4us/window, leaving the value at [:, :, 0].
  * BCE coefficients (s, w with q = p*w + s covering flip / rt>=4 / padding)
    prep on DVE under the gather shadow, Ln on ACT, fp32 reduces, and the
    host all-reduces the per-partition (sum_log_q, n_valid) pairs.
"""

import numpy as np

import concourse.bass as bass  # noqa: F401  (IndirectOffsetOnAxis unused now)
from concourse import bacc, mybir
from concourse.bass_utils import run_bass_kernel_spmd

# Problem geometry (hardcoded per contest contract).
B, N, M = 256, 512, 1024
NCORES = 8
BL = B // NCORES            # 32 batches per core
P = 128                     # SBUF partitions
TC = BL * M                 # 32768 claims per core
NW = 4                      # gather windows (8 batches each; int16 idx limit)
CW = TC // NW               # 8192 claims per window
GW = CW // P                # 64 claim groups per window
CF = NW * GW                # 256 claim columns in [128, 256] layout
NBLK = 32768                # 64-elem blocks per window
EPS = float(np.float32(1e-7))
ONE_M_EPS = float(np.float32(1.0 - 1e-7))

f32 = mybir.dt.float32
i32 = mybir.dt.int32
i16 = mybir.dt.int16
Alu = mybir.AluOpType
Act = mybir.ActivationFunctionType

_CACHE = {}


def _build_nc():
    nc = bacc.Bacc("TRN2", target_bir_lowering=False, debug=False,
                   dynamic_dma_scratch_size=65536)

    # Per-window adjacency shards: [32768 blocks, 64 floats].
    adjw = [nc.dram_tensor(f"adj{w}", [NBLK, 64], f32, kind="ExternalInput")
            for w in range(NW)]
    # Claim meta in window-slot layout [128, 4*CF]: vb | rt | tt | mk,
    # claim i of window w at (p = i%128, col = GW*w + i//128).
    claims = nc.dram_tensor("claims", [P, 4 * CF], i32, kind="ExternalInput")
    # va and vb2 in the SWDGE wrapped layout, host-replicated to all 8
    # GPSIMD core groups: claim i of window w at (16c + i%16, 128*w + i//16).
    wrap = nc.dram_tensor("wrap", [P, 2 * (TC // 16)], i32,
                          kind="ExternalInput")
    out = nc.dram_tensor("out", [P, 2], f32, kind="ExternalOutput")

    cl = nc.alloc_sbuf_tensor("cl", [P, 4 * CF], i32)
    wr = nc.alloc_sbuf_tensor("wr", [P, 2 * (TC // 16)], i32)
    t1 = nc.alloc_sbuf_tensor("t1", [P, TC // 16], i32)
    idxr = nc.alloc_sbuf_tensor("idxr", [P, TC // 16], i16)
    G = nc.alloc_sbuf_tensor("G", [P, NW * GW * 64], f32)
    praw = nc.alloc_sbuf_tensor("praw", [P, CF], f32)
    mb_t = nc.alloc_sbuf_tensor("mb_t", [P, 6 * CF], i32)
    s_tt = nc.alloc_sbuf_tensor("s_tt", [P, CF], f32)
    w_tt = nc.alloc_sbuf_tensor("w_tt", [P, CF], f32)
    q_t = nc.alloc_sbuf_tensor("q_t", [P, CF], f32)
    lg_t = nc.alloc_sbuf_tensor("lg_t", [P, CF], f32)
    ai_t = nc.alloc_sbuf_tensor("ai_t", [P, CF], i32)
    is4_t = nc.alloc_sbuf_tensor("is4_t", [P, CF], i32)
    vf_t = nc.alloc_sbuf_tensor("vf_t", [P, CF], f32)
    consts = nc.alloc_sbuf_tensor("consts", [P, 5], f32)  # .5, 1, 0, eps, 1-eps
    stats = nc.alloc_sbuf_tensor("stats", [P, 2], f32)    # [sum_log_q, n_valid]
    actwarm = nc.alloc_sbuf_tensor("actwarm", [P, 1], f32)

    s_wr = nc.alloc_semaphore("s_wr")       # wrap w0 slice DMA done
    s_wr2 = nc.alloc_semaphore("s_wr2")     # wrap rest DMA done
    s_cl = nc.alloc_semaphore("s_cl")       # claims DMA done
    s_idx = nc.alloc_semaphore("s_idx")     # idx16 ready (w0, then rest)
    s_g = [nc.alloc_semaphore(f"s_g{h}") for h in range(2 * NW)]
    s_vf = nc.alloc_semaphore("s_vf")       # BCE coeffs + n_valid ready
    s_q = nc.alloc_semaphore("s_q")         # per-window q ready
    s_lg = nc.alloc_semaphore("s_lg")       # per-window ln done
    s_ln = nc.alloc_semaphore("s_ln")       # final log reduce done
    s_out = nc.alloc_semaphore("s_out")     # output DMA done

    vb = cl.ap()[:, 0:CF]
    rt = cl.ap()[:, CF:2 * CF]
    tt = cl.ap()[:, 2 * CF:3 * CF]
    mk = cl.ap()[:, 3 * CF:4 * CF]
    vaw = wr.ap()[:, 0:TC // 16]
    vbw = wr.ap()[:, TC // 16:2 * (TC // 16)]

    # ---- input DMAs. Window 0's va/vb slices land first (sync queue) so
    # the first gather prep can start ~6us in; the rest follows on scalar.
    WC = TC // 16 // NW  # 512 wrapped cols per window
    nc.sync.dma_start(wr.ap()[:, 0:WC], wrap.ap()[:, 0:WC]).then_inc(s_wr, 16)
    nc.sync.dma_start(wr.ap()[:, 4 * WC:5 * WC],
                      wrap.ap()[:, 4 * WC:5 * WC]).then_inc(s_wr, 16)
    nc.scalar.dma_start(wr.ap()[:, WC:4 * WC],
                        wrap.ap()[:, WC:4 * WC]).then_inc(s_wr2, 16)
    nc.scalar.dma_start(wr.ap()[:, 5 * WC:8 * WC],
                        wrap.ap()[:, 5 * WC:8 * WC]).then_inc(s_wr2, 16)
    nc.scalar.dma_start(cl.ap()[:, :], claims.ap()[:, :]).then_inc(s_cl, 16)

    # ---- SCALAR: warm the Ln activation table while DMAs run ----
    nc.scalar.activation(out=actwarm.ap()[:, :], in_=actwarm.ap()[:, :],
                         func=Act.Ln, bias=1.0, scale=0.0)   # ln(0*x+1) = 0

    # ---- VECTOR: constants (no deps) ----
    nc.vector.memset(consts.ap()[:, 0:1], 0.5)
    nc.vector.memset(consts.ap()[:, 1:2], 1.0)
    nc.vector.memset(consts.ap()[:, 2:3], 0.0)
    nc.vector.memset(consts.ap()[:, 3:4], EPS)
    nc.vector.memset(consts.ap()[:, 4:5], ONE_M_EPS)

    # ---- VECTOR: gather indices. idx = va*8 + (vb2 >> 6), already in the
    # wrapped+replicated layout, written as int16. Window 0 first so the
    # gather pipeline starts early.
    for part, (lo, hi) in enumerate([(0, WC), (WC, 4 * WC)]):
        vas = wr.ap()[:, lo:hi]
        vbs = wr.ap()[:, 4 * WC + lo:4 * WC + hi]
        nc.vector.wait_ge(s_wr if part == 0 else s_wr2, 32)
        nc.vector.tensor_scalar(out=t1.ap()[:, lo:hi], in0=vbs,
                                scalar1=6, scalar2=None,
                                op0=Alu.arith_shift_right)
        nc.vector.drain()
        nc.vector.scalar_tensor_tensor(out=t1.ap()[:, lo:hi], in0=vas,
                                       scalar=8, in1=t1.ap()[:, lo:hi],
                                       op0=Alu.mult, op1=Alu.add)
        nc.vector.drain()
        nc.vector.tensor_scalar(out=idxr.ap()[:, lo:hi],
                                in0=t1.ap()[:, lo:hi],
                                scalar1=0, scalar2=None, op0=Alu.add)
        nc.vector.maybe_drain_then_inc((s_idx, 1))

    # ---- GPSIMD: four full-window gathers (8192 descriptors of 256B
    # each). Each dma_gather carries ~13us fixed ucode overhead on top of
    # ~6.2ns/descriptor, so fewer+bigger wins; w+2 gated on w's DMA done.
    for w in range(NW):
        nc.gpsimd.wait_ge(s_idx, 1 if w == 0 else 2)
        if w >= 2:
            nc.gpsimd.wait_ge(s_g[w - 2], 16)
        nc.gpsimd.dma_gather(
            out_ap=G.ap()[:, w * GW * 64:(w + 1) * GW * 64]
                    .rearrange("p (g e) -> p g e", e=64),
            in_ap=adjw[w].ap()[:, :],
            idxs_ap=idxr.ap()[:, w * (CW // 16):(w + 1) * (CW // 16)],
            num_idxs=CW,
            num_idxs_reg=CW,
            elem_size=64,
            transpose=False,
            single_packet=False,
        ).then_inc(s_g[w], 16)

    # ---- VECTOR: bit masks of c = vb & 63 (for the extraction selects)
    # and BCE coefficient prep, all under the gather shadow.
    nc.vector.wait_ge(s_cl, 16)
    for b in range(6):
        nc.vector.tensor_scalar(out=mb_t.ap()[:, b * CF:(b + 1) * CF],
                                in0=vb, scalar1=1 << b, scalar2=None,
                                op0=Alu.bitwise_and)
    nc.vector.tensor_scalar(out=ai_t.ap()[:, :], in0=rt, scalar1=1,
                            scalar2=None, op0=Alu.bitwise_and)
    nc.vector.tensor_scalar(out=is4_t.ap()[:, :], in0=rt, scalar1=4,
                            scalar2=None, op0=Alu.is_ge)
    nc.vector.tensor_scalar(out=vf_t.ap()[:, :], in0=mk, scalar1=0,
                            scalar2=None, op0=Alu.is_equal)
    nc.vector.drain()
    nc.vector.tensor_tensor(out=s_tt.ap()[:, :], in0=ai_t.ap()[:, :], in1=tt,
                            op=Alu.is_equal)
    nc.vector.tensor_reduce(out=stats.ap()[:, 1:2], in_=vf_t.ap()[:, :],
                            axis=mybir.AxisListType.X, op=Alu.add)
    nc.vector.drain()
    nc.vector.copy_predicated(out=s_tt.ap()[:, :], mask=is4_t.ap()[:, :],
                              data=consts.ap()[:, 0:1].to_broadcast([P, CF]))
    nc.vector.drain()
    nc.vector.tensor_scalar(out=w_tt.ap()[:, :], in0=s_tt.ap()[:, :],
                            scalar1=-2.0, scalar2=1.0,
                            op0=Alu.mult, op1=Alu.add)
    nc.vector.drain()
    nc.vector.copy_predicated(out=w_tt.ap()[:, :], mask=mk,
                              data=consts.ap()[:, 2:3].to_broadcast([P, CF]))
    nc.vector.copy_predicated(out=s_tt.ap()[:, :], mask=mk,
                              data=consts.ap()[:, 1:2].to_broadcast([P, CF]))
    nc.vector.maybe_drain_then_inc((s_vf, 1))

    # ---- VECTOR/SCALAR per window: extract claim element via 6 halving
    # selects on the bits of c, then q = p*w + s, clamp (via consts
    # tensors -- dual-immediate tensor_scalar stalls against SWDGE preps);
    # ACT does ln(q).
    for w in range(NW):
        G3 = G.ap()[:, w * GW * 64:(w + 1) * GW * 64] \
              .rearrange("p (g e) -> p g e", e=64)
        cols = slice(w * GW, (w + 1) * GW)
        nc.vector.wait_ge(s_g[w], 16)
        for b in range(5, -1, -1):
            hw = 1 << b
            mask = mb_t.ap()[:, b * CF + w * GW:b * CF + (w + 1) * GW] \
                       .unsqueeze(2).to_broadcast([P, GW, hw])
            nc.vector.copy_predicated(out=G3[:, :, 0:hw], mask=mask,
                                      data=G3[:, :, hw:2 * hw])
            nc.vector.drain()
        g0 = G3[:, :, 0:1].squeeze(2)
        if w == 0:
            nc.vector.wait_ge(s_vf, 1)
        nc.vector.tensor_tensor(out=q_t.ap()[:, cols], in0=g0,
                                in1=w_tt.ap()[:, cols], op=Alu.mult)
        nc.vector.drain()
        nc.vector.tensor_tensor(out=q_t.ap()[:, cols],
                                in0=q_t.ap()[:, cols],
                                in1=s_tt.ap()[:, cols], op=Alu.add)
        nc.vector.drain()
        nc.vector.tensor_tensor(out=q_t.ap()[:, cols],
                                in0=q_t.ap()[:, cols],
                                in1=consts.ap()[:, 3:4].to_broadcast([P, GW]),
                                op=Alu.max)
        nc.vector.drain()
        nc.vector.tensor_tensor(out=q_t.ap()[:, cols],
                                in0=q_t.ap()[:, cols],
                                in1=consts.ap()[:, 4:5].to_broadcast([P, GW]),
                                op=Alu.min)
        nc.vector.maybe_drain_then_inc((s_q, 1))

        nc.scalar.wait_ge(s_q, w + 1)
        nc.scalar.activation(out=lg_t.ap()[:, cols], in_=q_t.ap()[:, cols],
                             func=Act.Ln)
        nc.scalar.maybe_drain_then_inc((s_lg, 1))

    # ---- VECTOR: stats[:,0] = sum of ln(q) (full-fp32 DVE reduce) ----
    nc.vector.wait_ge(s_lg, NW)
    nc.vector.tensor_reduce(out=stats.ap()[:, 0:1], in_=lg_t.ap()[:, :],
                            axis=mybir.AxisListType.X, op=Alu.add)
    nc.vector.maybe_drain_then_inc((s_ln, 1))

    # ---- SYNC: ship per-partition stats; host does the tiny all-reduce ----
    nc.sync.wait_ge(s_ln, 1)
    nc.sync.wait_ge(s_vf, 1)
    nc.sync.dma_start(out.ap()[:, :], stats.ap()[:, :]).then_inc(s_out, 16)
    nc.sync.wait_ge(s_out, 16)

    nc.compile()
    return nc


def _prep_core_inputs(adj, va, vb, rt, tt, mk, c):
    """Build one core's input map (batches [32c, 32c+32))."""
    sl = slice(c * BL, (c + 1) * BL)
    adj_c = adj[sl].reshape(NW, NBLK, 64)

    def claim_layout(F):
        # claim i of window w -> (p = i%128, col = GW*w + i//128)
        X = F[sl].reshape(NW, GW, P)
        return np.concatenate([X[w].T for w in range(NW)], axis=1)

    def wrap_layout(F):
        # claim i of window w -> (i%16, 128*w + i//16), replicated x8
        X = F.reshape(NW, CW // 16, 16)
        W16 = np.concatenate([X[w].T for w in range(NW)], axis=1)
        return np.tile(W16, (8, 1))

    va_c = va[sl].reshape(TC)
    # vb2 folds the static window-local batch offset: claim i belongs to
    # local batch i//1024, i.e. batch (i//1024)%8 of its window.
    vb2_c = vb[sl].reshape(TC) + (1 << 18) * ((np.arange(TC) // M) % (BL // NW))

    in_map = {f"adj{w}": adj_c[w] for w in range(NW)}
    in_map["claims"] = np.concatenate(
        [claim_layout(vb), claim_layout(rt), claim_layout(tt),
         claim_layout(mk)], axis=1)
    in_map["wrap"] = np.concatenate(
        [wrap_layout(va_c), wrap_layout(vb2_c)], axis=1)
    return in_map


def kernel(posterior_adjacency, var_a, var_b, relation_type, is_true,
           claim_mask):
    adj = np.asarray(posterior_adjacency, dtype=np.float32)
    va = np.asarray(var_a, dtype=np.int32)
    vb = np.asarray(var_b, dtype=np.int32)
    rt = np.asarray(relation_type, dtype=np.int32)
    tt = np.asarray(is_true, dtype=np.int32)
    mk = np.asarray(claim_mask).astype(np.int32)

    if "nc" not in _CACHE:
        _CACHE["nc"] = _build_nc()
    nc = _CACHE["nc"]

    in_maps = [_prep_core_inputs(adj, va, vb, rt, tt, mk, c)
               for c in range(NCORES)]

    res = run_bass_kernel_spmd(nc, in_maps, core_ids=list(range(NCORES)))
    pairs = np.stack([r["out"] for r in res.results]).astype(np.float64)
    sum_log_q = pairs[:, :, 0].sum()
    n_valid = pairs[:, :, 1].sum()
    if n_valid > 0:
        loss = -sum_log_q / max(n_valid, 1.0)
    else:
        loss = 0.0
    return np.float32(loss)
